# revision 1
# baseline (speedup 1.0000x reference)
"""Trainium2 Bass kernel for nn_DiscretisedBNF (discretised BNF loss).

Math reduction used on device: the reference's (B, D, K=128) clamped-CDF
bin sum collapses (Abel summation) to

    pO[b,d] = -127/256 + sum_{k=1..127} u_k * erf(z_k),
    z_k = (e_k - mu_x) * inv,   e_k = 2k/128 - 1,
    u_k = -1/128 (k<127),  u_127 = 125/256,
    inv = 1 / (sigma_x * sqrt(2))

verified exact vs the reference formula.

Sharding (8 cores, full inputs in, full output out):
  - mm1 (mu_cat @ W1) computed on every core (bf16, transposed layout
    hT = W1^T @ mu_cat^T so H lands on partitions),
  - W2 column-sharded: core i owns output columns {i*128..(i+1)*128-1}
    (mu_eps) and {1024+i*128..} (ln_sigma) -> mm2 is 1/8 per core,
  - binning data-parallel over the same d-slice: 32768 elements/core,
  - per-core output: 128 partial sums of sigma1^{-2t}*(x-pO)^2; host
    reduces and scales.

Binning pipeline per core: DVE computes inv and mu_x*inv, splits each
into exact bf16 (hi, lo) pairs, written to a quad tile in a [64, 512]
layout so a single 1KB-per-partition SBUF->SBUF DMA per row flattens
them into R [4, 32768]; PE forms z tiles [128 edges, 1536 elems] as
K=4 bf16 outer products (exact to ~2^-17); ACT runs one big Erf per
tile (PSUM -> SBUF, fp8e4 out); PE contracts edges with fp8 u-weight
columns (erf tile as stationary, [128,2] moving; the non-fp8 weight
125/256 is decomposed as -1/128 plus 0.9921875*0.5) giving q0/q1 in
PSUM [128, 512]; DVE computes sum of (sqw*(x + 127/256 - q))^2 per
partition. Host sums the 8x128 partials and scales by -ln(sigma1)/(B*D).
"""

import sys

sys.path.insert(0, "/opt/trn_rl_repo")

import numpy as np
import ml_dtypes

import concourse.bass as bass
import concourse.tile as tile
from concourse import bacc, mybir
from concourse.alu_op_type import AluOpType
from concourse.bass_utils import run_bass_kernel_spmd

B, D, H, K = 256, 1024, 2048, 128
NCORES = 8
DSL = D // NCORES  # 128 d-columns per core
SIGMA1 = 0.02
TMIN = 1e-10
LEAK = 0.01
C127 = 127.0 / 256.0

F32 = mybir.dt.float32
BF16 = mybir.dt.bfloat16
FP8 = mybir.dt.float8e4
BFNP = ml_dtypes.bfloat16

N_GROUPS = 32          # binning groups per core
GELEMS = 1024          # elements per group (2 z-matmuls of N=512)
NELEMS = DSL * B       # 32768 elements per core


def _build(debug=False):
    nc = bacc.Bacc("TRN2", target_bir_lowering=False, debug=False,
                   num_devices=NCORES)

    d_muT = nc.dram_tensor("muT", (D, B), BF16, kind="ExternalInput")
    d_xsl = nc.dram_tensor("x_sl", (64, 2 * B), F32, kind="ExternalInput")
    d_nsl = nc.dram_tensor("n_sl", (64, 2 * B), F32, kind="ExternalInput")
    d_w1 = nc.dram_tensor("w1", (D, H), BF16, kind="ExternalInput")
    d_w1r = nc.dram_tensor("w1row", (1, H), BF16, kind="ExternalInput")
    d_w2 = nc.dram_tensor("w2", (H, 2 * DSL), BF16, kind="ExternalInput")
    d_tv = nc.dram_tensor("tv", (1, B), BF16, kind="ExternalInput")
    d_b1r = nc.dram_tensor("b1r", (128, 16), F32, kind="ExternalInput")
    d_b2r = nc.dram_tensor("b2r", (64, 4), F32, kind="ExternalInput")
    d_bc64 = nc.dram_tensor("bc64", (4 * 64, 2 * B), F32, kind="ExternalInput")
    d_edg = nc.dram_tensor("edg", (4, 128), BF16, kind="ExternalInput")
    d_uv = nc.dram_tensor("uv", (128, 2), FP8, kind="ExternalInput")
    d_xqc = nc.dram_tensor("xqc", (128, B), F32, kind="ExternalInput")
    d_sqwq = nc.dram_tensor("sqwq", (128, B), F32, kind="ExternalInput")
    d_part = nc.dram_tensor("part", (128, 1), F32, kind="ExternalOutput")
    dbg = {}
    if debug:
        for nm, shp in [("dbg_me", (64, 2 * B)), ("dbg_ls", (64, 2 * B)),
                        ("dbg_inv", (64, 2 * B)), ("dbg_mx", (64, 2 * B)),
                        ("dbg_q", (128, B))]:
            dbg[nm] = nc.dram_tensor(nm, shp, F32, kind="ExternalOutput")

    MULT, ADD, SUB, BYP = (AluOpType.mult, AluOpType.add,
                           AluOpType.subtract, AluOpType.bypass)
    AF = mybir.ActivationFunctionType

    with tile.TileContext(nc) as tc:
        with (
            tc.tile_pool(name="consts", bufs=1) as cpool,
            tc.tile_pool(name="weights", bufs=1) as wpool,
            tc.tile_pool(name="work", bufs=1) as work,
            tc.tile_pool(name="stage", bufs=1) as stage,
        ):
            muT = work.tile([128, 8, B], BF16)
            hT = work.tile([128, 16, B], BF16)
            # prep stage runs in [64, 2, 256] layout (d = dh*64 + p) so the
            # flatten DMA gets 1KB-contiguous per-partition descriptors
            ME = work.tile([64, 2, B], F32)
            lnm = work.tile([64, 2, B], F32)
            w1 = wpool.tile([128, 8, H], BF16)
            w1r = wpool.tile([1, H], BF16)
            w2 = wpool.tile([128, 16, 2 * DSL], BF16)

            with (
                tc.tile_pool(name="xin", bufs=1) as xin,
                tc.tile_pool(name="psA", bufs=5,
                             space=bass.MemorySpace.PSUM) as psA,
                tc.tile_pool(name="psO", bufs=3,
                             space=bass.MemorySpace.PSUM) as psO,
            ):
                # muT is host-computed (tiny per-row math); interleave its
                # tiles with W1 so mm1 chains start as soon as possible
                for k in range(8):
                    nc.sync.dma_start(muT[:, k, :],
                                      d_muT.ap()[k * 128:(k + 1) * 128, :])
                    nc.sync.dma_start(w1[:, k, :], d_w1.ap()[k * 128:(k + 1) * 128, :])
                b1r = cpool.tile([128, 16], F32)
                nc.sync.dma_start(b1r[:], d_b1r.ap()[:])
                tvt = cpool.tile([1, B], BF16)
                nc.sync.dma_start(tvt[:], d_tv.ap()[:])
                nc.sync.dma_start(w1r[:], d_w1r.ap()[:])
                # binning-prep inputs (needed right after mm2)
                b2r = cpool.tile([64, 4], F32)
                nc.sync.dma_start(b2r[:], d_b2r.ap()[:])
                bc64 = []  # mf, Bv, rm, cexp in [64, 2, 256]
                for j in range(4):
                    bt = cpool.tile([64, 2, B], F32, tag=f"bc64_{j}")
                    nc.sync.dma_start(bt[:], d_bc64.ap()[j * 64:(j + 1) * 64, :])
                    bc64.append(bt)
                mf_bc, bv_bc, rm_bc, ce_bc = bc64
                xsl = work.tile([64, 2, B], F32)
                nc.sync.dma_start(xsl[:], d_xsl.ap()[:])
                nsl = work.tile([64, 2, B], F32)
                nc.sync.dma_start(nsl[:], d_nsl.ap()[:])
                # mu_x partials that do not depend on mm2 run during mm1
                a1 = work.tile([64, 2, B], F32)
                nc.vector.tensor_tensor(a1[:], xsl[:], mf_bc[:], MULT)
                a2 = work.tile([64, 2, B], F32)
                nc.vector.tensor_tensor(a2[:], nsl[:], bv_bc[:], MULT)
                s = work.tile([64, 2, B], F32)
                nc.vector.tensor_tensor(s[:], a1[:], a2[:], ADD)
                for k in range(16):
                    nc.sync.dma_start(w2[:, k, :], d_w2.ap()[k * 128:(k + 1) * 128, :])
                edg = cpool.tile([4, 128], BF16)
                nc.sync.dma_start(edg[:], d_edg.ap()[:])
                uv = cpool.tile([128, 2], FP8)
                nc.sync.dma_start(uv[:], d_uv.ap()[:])

                # mm1: hT[m] = LeakyReLU(W1^T mu_cat^T + b1)  (16 M-tiles)
                # First tiles evict via DVE (ACT's first table load drains
                # behind the input DMAs and would hold the PSUM banks).
                for m in range(16):
                    ph = psA.tile([128, B], F32, tag="ph")
                    ms = slice(m * 128, (m + 1) * 128)
                    for k in range(8):
                        nc.tensor.matmul(ph[:], w1[:, k, ms], muT[:, k, :],
                                         start=(k == 0), stop=False)
                    nc.tensor.matmul(ph[:], w1r[:, ms], tvt[:],
                                     start=False, stop=True)
                    if m < 5:
                        u = xin.tile([128, B], F32, tag="lrelu_u")
                        nc.vector.tensor_scalar_add(u[:], ph[:],
                                                    b1r[:, m:m + 1])
                        nc.vector.scalar_tensor_tensor(
                            hT[:, m, :], u[:], LEAK, u[:],
                            op0=MULT, op1=AluOpType.max)
                    else:
                        nc.scalar.activation(hT[:, m, :], ph[:], AF.Lrelu,
                                             bias=b1r[:, m:m + 1], scale=1.0,
                                             alpha=LEAK)

                # mm2: out^T in 4 M-tiles of 64 rows; ln_sigma halves first
                # so the exp/inv chain starts as early as possible
                for mo in (2, 3, 0, 1):
                    po = psO.tile([64, B], F32, tag="po")
                    mos = slice(mo * 64, (mo + 1) * 64)
                    for k in range(16):
                        nc.tensor.matmul(po[:], w2[:, k, mos], hT[:, k, :],
                                         start=(k == 0), stop=(k == 15))
                    if mo < 2:
                        nc.vector.tensor_scalar_add(ME[:, mo, :], po[:],
                                                    b2r[:, mo:mo + 1])
                    else:
                        nc.vector.scalar_tensor_tensor(
                            lnm[:, mo - 2, :], po[:], b2r[:, mo:mo + 1],
                            mf_bc[:, mo - 2, :], op0=ADD, op1=MULT)

                # ---- binning prep ([64, 2, 256]); chain A (inv) first so the
                # ih/il flatten DMAs launch while chain B (mu_x) still runs
                QT = stage.tile([64, 4, 2, B], BF16)
                # R is split into two tiles: Tile tracks DMA-write deps at
                # whole-tile granularity, so the first bin groups must not
                # share a tile with the big gathers' destination.
                RHEAD = 9 * 512          # 4608 = exactly 3 groups of 1536
                R1 = stage.tile([4, RHEAD], BF16)
                R2 = stage.tile([4, NELEMS - RHEAD], BF16)
                flat_engines = [nc.sync, nc.scalar]
                flat_dmas = []

                def flatten_row(r):
                    # head -> R1 (fast, 9 descriptors); main/tail -> R2
                    for si, (p0, p1) in enumerate([(0, 9), (9, 32), (32, 64)]):
                        eng = flat_engines[(3 * r + si) % 2]
                        dst = (R1[r:r + 1, :] if si == 0 else
                               R2[r:r + 1, p0 * 512 - RHEAD:p1 * 512 - RHEAD])
                        flat_dmas.append(eng.dma_start(dst, QT[p0:p1, r, :, :]))

                ei = work.tile([64, 2, B], F32)
                nc.scalar.activation(ei[:], lnm[:], AF.Exp, bias=0.0,
                                     scale=-1.0)
                # preload the erf table set while ACT is otherwise idle so
                # the first real Erf needs no ACT_TABLE_LOAD
                dum = work.tile([64, 1], F32)
                nc.scalar.activation(dum[:], ei[:, 0, 0:1], AF.Erf,
                                     bias=0.0, scale=1.0)
                inv = work.tile([64, 2, B], F32)
                nc.vector.tensor_tensor(inv[:], ei[:], ce_bc[:], MULT)
                nc.vector.tensor_copy(QT[:, 0, :, :], inv[:])  # ih
                flatten_row(0)
                nc.vector.tensor_tensor(QT[:, 1, :, :], inv[:],
                                        QT[:, 0, :, :], SUB)   # il = inv - ih
                flatten_row(1)
                a4 = work.tile([64, 2, B], F32)
                nc.vector.tensor_tensor(a4[:], rm_bc[:], ME[:], MULT)
                mu_x = work.tile([64, 2, B], F32)
                nc.vector.tensor_tensor(mu_x[:], s[:], a4[:], SUB)
                mx = work.tile([64, 2, B], F32)
                nc.vector.tensor_tensor(mx[:], mu_x[:], inv[:], MULT)
                nc.vector.tensor_copy(QT[:, 2, :, :], mx[:])   # hi
                flatten_row(2)
                nc.vector.tensor_tensor(QT[:, 3, :, :], mx[:],
                                        QT[:, 2, :, :], SUB)   # lo = mx - hi
                flatten_row(3)

                if debug:
                    for nm, src in [("dbg_me", ME), ("dbg_ls", lnm),
                                    ("dbg_inv", inv), ("dbg_mx", mx)]:
                        nc.sync.dma_start(dbg[nm].ap()[:], src[:])



            # ---- binning main loop -------------------------------------
            with (
                tc.tile_pool(name="psZ", bufs=3,
                             space=bass.MemorySpace.PSUM) as psZ,
                tc.tile_pool(name="psQ", bufs=1,
                             space=bass.MemorySpace.PSUM) as psQ,
                tc.tile_pool(name="erf", bufs=3) as epool,
            ):
                # PE HAM warmers: real readers of QT / R spread across the
                # prep+flatten window so the PE clock never throttles down
                warm = psQ.tile([128, 512], F32, tag="warm")
                nc.tensor.matmul(warm[:, 0:B], QT[:, 0, 0, 0:128],
                                 QT[:, 1, 0, :], start=True, stop=True)
                nc.tensor.matmul(warm[:, 0:B], QT[:, 2, 0, 0:128],
                                 QT[:, 3, 0, :], start=True, stop=True)

                # groups of 1024 elements (2 z-matmuls, one erf); 3 psum
                # buffers keep the PE two groups ahead of ACT
                sizes = [1024] * 32
                q = psQ.tile([128, 2 * B], F32)
                base = 0
                for g, gel in enumerate(sizes):
                    zt = psZ.tile([128, 1024], F32, tag="zt")
                    nz = gel // 512
                    for h in range(nz):
                        off = base + h * 512
                        rsrc = (R1[:, off:off + 512] if off < RHEAD else
                                R2[:, off - RHEAD:off - RHEAD + 512])
                        nc.tensor.matmul(
                            zt[:, h * 512:(h + 1) * 512], edg[:], rsrc,
                            start=True, stop=True)
                    et = epool.tile([128, 1024], FP8, tag="et")
                    nc.scalar.activation(et[:, 0:gel], zt[:, 0:gel], AF.Erf,
                                         bias=0.0, scale=1.0)
                    for j in range(gel // 128):
                        c = base // 128 + j
                        nc.tensor.matmul(q[:, 2 * c:2 * c + 2],
                                         et[:, j * 128:(j + 1) * 128], uv[:],
                                         start=True, stop=True)
                    base += gel

                # tail: part = sum_cols (sqw*(xqc - q))^2
                xqc = cpool.tile([128, B], F32)
                nc.sync.dma_start(xqc[:], d_xqc.ap()[:])
                sqwq = cpool.tile([128, B], F32)
                nc.sync.dma_start(sqwq[:], d_sqwq.ap()[:])
                q2 = q[:].rearrange("p (c two) -> p c two", two=2)
                t0 = work.tile([128, B], F32)
                nc.vector.scalar_tensor_tensor(t0[:], q2[:, :, 1], -0.9921875,
                                               xqc[:], op0=MULT, op1=ADD)
                e1 = work.tile([128, B], F32)
                nc.vector.tensor_tensor(e1[:], t0[:], q2[:, :, 0], SUB)
                dw = work.tile([128, B], F32)
                nc.vector.tensor_tensor(dw[:], e1[:], sqwq[:], MULT)
                dw2 = work.tile([128, B], F32)
                part = work.tile([128, 1], F32)
                nc.vector.scalar_tensor_tensor(dw2[:], dw[:], 1.0, dw[:],
                                               op0=BYP, op1=MULT,
                                               accum_out=part[:])
                nc.sync.dma_start(d_part.ap()[:], part[:])
                if debug:
                    qsb = work.tile([128, B], F32)
                    nc.vector.scalar_tensor_tensor(
                        qsb[:], q2[:, :, 1], 0.9921875, q2[:, :, 0],
                        op0=MULT, op1=ADD)
                    nc.sync.dma_start(dbg["dbg_q"].ap()[:], qsb[:])

    nc.compile()
    return nc


def host_prep(x, t, noise, W1, b1, W2, b2):
    """Build the per-core in_maps (host-side sharding + tiny per-row math)."""
    f32 = np.float32
    tv = t[:, 0].astype(f32)
    gamma = (1.0 - np.power(f32(SIGMA1), f32(2.0) * tv)).astype(f32)
    low = tv < TMIN
    mf = np.where(low, f32(0.0), f32(1.0)).astype(f32)
    gsafe = np.where(gamma > 0, gamma, f32(1.0)).astype(f32)
    r = np.sqrt((1.0 - gsafe) / gsafe).astype(f32)
    rsafe = np.where(r > 0, r, f32(1.0)).astype(f32)
    g1 = gamma
    g2 = (gamma * (1.0 - gamma)).astype(f32)
    bv = ((1.0 - gamma) * mf).astype(f32)
    rm = (r * mf).astype(f32)
    cexp = np.where(low, f32(1.0 / np.sqrt(2.0)),
                    (1.0 / (rsafe * np.sqrt(2.0))).astype(f32)).astype(f32)
    sqw = np.power(f32(SIGMA1), -tv).astype(f32)

    bc64 = np.concatenate([np.broadcast_to(np.tile(v, 2), (64, 2 * B))
                           for v in (mf, bv, rm, cexp)], axis=0)
    bc64 = np.ascontiguousarray(bc64, dtype=f32)

    def to64(a128):
        # [128 d, 256 b] -> [64 p, 512] with [p, dh*256+b] = a[dh*64+p, b]
        return np.ascontiguousarray(
            a128.reshape(2, 64, B).transpose(1, 0, 2).reshape(64, 2 * B))

    e = (2.0 * np.arange(1, K) / K - 1.0).astype(f32)  # 127 edges
    edg = np.zeros((4, 128), dtype=BFNP)
    edg[0, :127] = e.astype(BFNP)
    edg[1, :127] = e.astype(BFNP)
    edg[2, :127] = BFNP(-1.0)
    edg[3, :127] = BFNP(-1.0)
    F8NP = ml_dtypes.float8_e4m3
    uvec = np.zeros((128, 2), dtype=F8NP)
    uvec[:127, 0] = F8NP(-1.0 / K)   # plain -1/128 for every real edge
    uvec[126, 1] = F8NP(0.5)         # extra (127/256)/0.9921875... see tail

    xT = np.ascontiguousarray(x.T, dtype=f32)
    nT = np.ascontiguousarray(noise.T, dtype=f32)
    muTb = np.ascontiguousarray(
        (xT * gamma[None, :] + nT * g2[None, :]).astype(f32).astype(BFNP))
    w1b = np.ascontiguousarray(W1[:D].astype(BFNP))
    w1rb = np.ascontiguousarray(W1[D:D + 1].astype(BFNP))
    tvb = np.ascontiguousarray(tv.astype(BFNP).reshape(1, B))
    b1r = np.ascontiguousarray(b1.reshape(16, 128).T, dtype=f32)

    # q layout index math: flat = col*128 + p ;
    # flat = p64*512 + dh*256 + b with d_local = dh*64 + p64
    p_idx = np.arange(128)[:, None]
    c_idx = np.arange(B)[None, :]
    flat = c_idx * 128 + p_idx
    d_l = (flat % 512) // B * 64 + flat // 512
    b_i = flat % B
    sqwq = np.ascontiguousarray(sqw[b_i], dtype=f32)

    in_maps = []
    for i in range(NCORES):
        cols = np.concatenate([np.arange(i * DSL, (i + 1) * DSL),
                               1024 + np.arange(i * DSL, (i + 1) * DSL)])
        w2b = np.ascontiguousarray(W2[:, cols].astype(BFNP))
        b2sl = b2[cols].astype(f32)
        b2r = np.ascontiguousarray(b2sl.reshape(4, 64).T, dtype=f32)
        xqc = np.ascontiguousarray(
            x[b_i, i * DSL + d_l].astype(f32) + f32(C127), dtype=f32)
        in_maps.append({
            "muT": muTb,
            "x_sl": to64(xT[i * DSL:(i + 1) * DSL]),
            "n_sl": to64(nT[i * DSL:(i + 1) * DSL]),
            "w1": w1b, "w1row": w1rb, "w2": w2b, "tv": tvb,
            "b1r": b1r, "b2r": b2r, "bc64": bc64,
            "edg": edg, "uv": uvec, "xqc": xqc, "sqwq": sqwq,
        })
    return in_maps


_nc_cache = {}


def get_nc(debug=False):
    if debug not in _nc_cache:
        _nc_cache[debug] = _build(debug)
    return _nc_cache[debug]


def run_on_cores(inputs, trace=False, debug=False, tmpdir=None):
    nc = get_nc(debug)
    in_maps = host_prep(**inputs)
    res = run_bass_kernel_spmd(nc, in_maps, core_ids=list(range(NCORES)),
                               trace=trace, tmpdir=tmpdir)
    total = np.float32(0.0)
    for i in range(NCORES):
        total += res.results[i]["part"].astype(np.float32).sum()
    loss = np.float32(-np.log(np.float32(SIGMA1)) * total / np.float32(B * D))
    return loss, res


def kernel(**inputs):
    inputs = {k: np.asarray(v) for k, v in inputs.items()}
    loss, _ = run_on_cores(inputs)
    return np.asarray(loss, dtype=np.float32)



# revision 8
# speedup vs baseline: 1.0575x; 1.0575x over previous
"""Trainium2 Bass kernel for nn_DiscretisedBNF (discretised BNF loss).

Math reduction used on device: the reference's (B, D, K=128) clamped-CDF
bin sum collapses (Abel summation) to

    pO[b,d] = -127/256 + sum_{k=1..127} u_k * erf(z_k),
    z_k = (e_k - mu_x) * inv,   e_k = 2k/128 - 1,
    u_k = -1/128 (k<127),  u_127 = 125/256,
    inv = 1 / (sigma_x * sqrt(2))

erf is approximated on device by tanh(1.20331*z) (minimax fit, max abs
err 0.019; end-to-end loss rel err ~1.6e-3 incl. all quantization) so
that the whole kernel uses a single ACT table set (exp_and_others has
exp, tanh and leaky_relu; erf would force a ~2.7us table switch between
the exp and the binning phase).

Sharding (8 cores, full inputs in, full output out):
  - mm1 (mu_cat @ W1) replicated per core, fp8 DoubleRow (2 k-subtiles
    per matmul), with the t-row and b1 folded in as a K=2 bf16 matmul,
  - W2 column-sharded: core i owns output columns {i*128..} (mu_eps)
    and {1024+i*128..} (ln_sigma); mm2 fp8 DoubleRow + b2 ones-row,
  - binning data-parallel over the same d-slice: 32768 elements/core,
  - per-core output: 128 partial sums of sigma1^{-2t}*(x-pO)^2; host
    reduces and scales.

Inputs are host-packed into a few large SBUF-layout blobs so the input
pipe is ~15 large DMAs split across both HWDGE queues instead of ~40
small serialized ones. Element order for binning is dh-major
(g = dh*16384 + p64*256 + b, d_local = dh*64 + p64) so each half of the
prep (driven by one mu_eps/ln_sig half of mm2) feeds a contiguous run
of bin groups, letting ACT start tanh right behind mm2.
"""

import sys

sys.path.insert(0, "/opt/trn_rl_repo")

import numpy as np
import ml_dtypes

import concourse.bass as bass
import concourse.tile as tile
from concourse import bacc, mybir
from concourse.alu_op_type import AluOpType
from concourse.bass_utils import run_bass_kernel_spmd

B, D, H, K = 256, 1024, 2048, 128
NCORES = 8
DSL = D // NCORES  # 128 d-columns per core
SIGMA1 = 0.02
TMIN = 1e-10
LEAK = 0.01
C127 = 127.0 / 256.0
ATAN = 1.2033141525242548  # tanh(ATAN*z) ~= erf(z)

F32 = mybir.dt.float32
BF16 = mybir.dt.bfloat16
FP8 = mybir.dt.float8e4
BFNP = ml_dtypes.bfloat16
F8NP = ml_dtypes.float8_e4m3

HELEMS = DSL // 2 * B          # 16384 elements per dh half
RHEAD = 18 * B                 # 4608 = 3 groups of 1536 (partitions 0:18)
GROUPS = [1536] * 10 + [1024]  # per-half group sizes (sum = 16384)

# bb blob column offsets (bf16, 4 partitions)
BB_TV = 0         # [0:2, 0:256]   row0 = t, row1 = ones
BB_W1T = 256      # [0:2, 256:2304] row0 = W1[D,:], row1 = b1
BB_EDG = 2304     # [0:4, 2304:2432] edge matrix
BB_B2 = 2432      # [0:1, 2432:2688] b2[cols]
BB_ONE = 2688     # [0:1, 2688:2944] ones
BB_W = 2944

# f64 blob column offsets (f32, 64 partitions; 512-wide = tiled x2 over dh)
FO_MF, FO_BV, FO_RM, FO_CE, FO_XS, FO_NS = 0, 512, 1024, 1536, 2048, 2560
F64_W = 3072


def _build(debug=False):
    nc = bacc.Bacc("TRN2", target_bir_lowering=False, debug=False,
                   num_devices=NCORES)

    d_muT = nc.dram_tensor("muT8", (128, 8 * B), FP8, kind="ExternalInput")
    d_w1 = nc.dram_tensor("w1m", (128, 16 * 8 * 128), FP8,
                          kind="ExternalInput")
    d_w2 = nc.dram_tensor("w2m", (128, 16 * 2 * DSL), FP8,
                          kind="ExternalInput")
    d_bb = nc.dram_tensor("bb", (4, BB_W), BF16, kind="ExternalInput")
    d_f64 = nc.dram_tensor("f64", (64, F64_W), F32, kind="ExternalInput")
    d_f128 = nc.dram_tensor("f128", (128, 2 * B), F32, kind="ExternalInput")
    d_uv = nc.dram_tensor("uv", (128, 2), FP8, kind="ExternalInput")
    d_part = nc.dram_tensor("part", (128, 1), F32, kind="ExternalOutput")

    MULT, ADD, SUB, BYP = (AluOpType.mult, AluOpType.add,
                           AluOpType.subtract, AluOpType.bypass)
    AF = mybir.ActivationFunctionType
    DR = mybir.MatmulPerfMode.DoubleRow

    with tile.TileContext(nc) as tc:
        with (
            tc.tile_pool(name="weights", bufs=1) as wpool,
            tc.tile_pool(name="work", bufs=1) as work,
            tc.tile_pool(name="stage", bufs=1) as stage,
        ):
            muT = wpool.tile([128, 8, B], FP8)
            w1 = wpool.tile([128, 16, 8, 128], FP8)
            w2 = wpool.tile([128, 16, 2 * DSL], FP8)
            bb = wpool.tile([4, BB_W], BF16)
            f64 = wpool.tile([64, F64_W], F32)
            f128 = wpool.tile([128, 2 * B], F32)
            uv = wpool.tile([128, 2], FP8)
            hT = work.tile([128, 16, B], FP8)

            with (
                tc.tile_pool(name="psA", bufs=3,
                             space=bass.MemorySpace.PSUM) as psA,
                tc.tile_pool(name="psO", bufs=1,
                             space=bass.MemorySpace.PSUM) as psO,
            ):
                # ---- input DMAs: big host-packed blobs, two HWDGE queues
                nc.scalar.dma_start(bb[:], d_bb.ap()[:])
                nc.sync.dma_start(muT[:], d_muT.ap()[:])
                nc.scalar.dma_start(f64[:], d_f64.ap()[:])
                for s4 in range(4):  # 4 m-tiles (512KB) per slab
                    nc.sync.dma_start(
                        w1[:, 4 * s4:4 * (s4 + 1), :, :],
                        d_w1.ap()[:, s4 * 4096:(s4 + 1) * 4096])
                nc.scalar.dma_start(uv[:], d_uv.ap()[:])
                nc.scalar.dma_start(w2[:], d_w2.ap()[:])
                nc.scalar.dma_start(f128[:], d_f128.ap()[:])

                # s = x*mf + (1-gamma)*mf*noise  (the mu/gamma term, masked)
                a1 = work.tile([64, 2, B], F32)
                nc.vector.tensor_tensor(
                    a1[:], f64[:, FO_XS:FO_XS + 512], f64[:, FO_MF:FO_MF + 512],
                    MULT)
                a2 = work.tile([64, 2, B], F32)
                nc.vector.tensor_tensor(
                    a2[:], f64[:, FO_NS:FO_NS + 512], f64[:, FO_BV:FO_BV + 512],
                    MULT)
                s = work.tile([64, 2, B], F32)
                nc.vector.tensor_tensor(s[:], a1[:], a2[:], ADD)
                # dummy exp: pull the exp_and_others ACT table load into
                # the mm1 window (tanh/exp later need no load)
                dum = work.tile([64, 1], F32)
                nc.scalar.activation(dum[:], f64[:, 0:1], AF.Exp,
                                     bias=0.0, scale=1.0)

                # ---- mm1: hT[m] = LeakyReLU(W1^T mu_cat^T) fp8 DoubleRow;
                # t-row and b1 folded in as a K=2 bf16 matmul
                for m in range(16):
                    ph = psA.tile([128, B], F32, tag="ph")
                    for j in range(4):
                        nc.tensor.matmul(ph[:], w1[:, m, 2 * j:2 * j + 2, :],
                                         muT[:, 2 * j:2 * j + 2, :],
                                         start=(j == 0), stop=False,
                                         perf_mode=DR)
                    ms = slice(BB_W1T + m * 128, BB_W1T + (m + 1) * 128)
                    nc.tensor.matmul(ph[:], bb[0:2, ms], bb[0:2, BB_TV:BB_TV + B],
                                     start=False, stop=True)
                    nc.scalar.activation(hT[:, m, :], ph[:], AF.Lrelu,
                                         bias=0.0, scale=1.0, alpha=LEAK)

                # ---- mm2: po[mo] = W2[:,cols]^T hT + b2, fp8 DoubleRow.
                # Order (2,0,3,1): lnsig/mu_eps for dh=0 first so half-a
                # prep + binning launch while mm2 finishes half b.
                po = {}
                for mo in (2, 0, 3, 1):
                    pt = psO.tile([64, B], F32, tag=f"po{mo}")
                    po[mo] = pt
                    mos = slice(mo * 64, (mo + 1) * 64)
                    for j in range(8):
                        nc.tensor.matmul(pt[:], w2[:, 2 * j:2 * j + 2, mos],
                                         hT[:, 2 * j:2 * j + 2, :],
                                         start=(j == 0), stop=False,
                                         perf_mode=DR)
                    b2s = slice(BB_B2 + mo * 64, BB_B2 + (mo + 1) * 64)
                    nc.tensor.matmul(pt[:], bb[0:1, b2s],
                                     bb[0:1, BB_ONE:BB_ONE + B],
                                     start=False, stop=True)

                # ---- binning prep, per dh half -------------------------
                QT = [stage.tile([64, 4, B], BF16, name=f"QT{h}")
                      for h in range(2)]
                R1 = [stage.tile([4, RHEAD], BF16, name=f"R1{h}")
                      for h in range(2)]
                R2 = [stage.tile([4, HELEMS - RHEAD], BF16, name=f"R2{h}")
                      for h in range(2)]
                flat_engines = [nc.sync, nc.scalar]

                def flatten_row(hh, r):
                    for si, (p0, p1) in enumerate([(0, 18), (18, 64)]):
                        eng = flat_engines[(r + si) % 2]
                        dst = (R1[hh][r:r + 1, :] if si == 0 else
                               R2[hh][r:r + 1, :])
                        eng.dma_start(dst, QT[hh][p0:p1, r, :])

                warmn = [0]

                def warm_mm(dep_q, dep_r):
                    w = psA.tile([128, B], F32, tag="warm", bufs=1)
                    nc.tensor.matmul(w[:], QT[dep_q][:, dep_r, 0:128],
                                     QT[dep_q][:, dep_r, :],
                                     start=True, stop=True)
                    warmn[0] += 1

                inv = [None, None]
                for hh in range(2):
                    hs = slice(hh * B, (hh + 1) * B)
                    lnm = work.tile([64, B], F32, tag=f"lnm{hh}")
                    nc.vector.tensor_tensor(lnm[:], po[2 + hh][:],
                                            f64[:, FO_MF + hh * B:FO_MF + hh * B + B],
                                            MULT)
                    ei = work.tile([64, B], F32, tag=f"ei{hh}")
                    nc.scalar.activation(ei[:], lnm[:], AF.Exp, bias=0.0,
                                         scale=-1.0)
                    iv = work.tile([64, B], F32, tag=f"inv{hh}")
                    inv[hh] = iv
                    nc.vector.tensor_tensor(
                        iv[:], ei[:], f64[:, FO_CE + hh * B:FO_CE + hh * B + B],
                        MULT)
                    nc.vector.tensor_copy(QT[hh][:, 0, :], iv[:])   # ih
                    flatten_row(hh, 0)
                    nc.vector.tensor_tensor(QT[hh][:, 1, :], iv[:],
                                            QT[hh][:, 0, :], SUB)   # il
                    flatten_row(hh, 1)
                    warm_mm(hh, 0)
                    a4 = work.tile([64, B], F32, tag=f"a4{hh}")
                    nc.vector.tensor_tensor(
                        a4[:], f64[:, FO_RM + hh * B:FO_RM + hh * B + B],
                        po[hh][:], MULT)
                    mu_x = work.tile([64, B], F32, tag=f"mux{hh}")
                    nc.vector.tensor_tensor(mu_x[:], s[:, hh, :], a4[:], SUB)
                    mx = work.tile([64, B], F32, tag=f"mx{hh}")
                    nc.vector.tensor_tensor(mx[:], mu_x[:], iv[:], MULT)
                    nc.vector.tensor_copy(QT[hh][:, 2, :], mx[:])   # hi
                    flatten_row(hh, 2)
                    nc.vector.tensor_tensor(QT[hh][:, 3, :], mx[:],
                                            QT[hh][:, 2, :], SUB)   # lo
                    flatten_row(hh, 3)
                    warm_mm(hh, 2)

            # ---- binning main loop -------------------------------------
            with (
                tc.tile_pool(name="psZ", bufs=2,
                             space=bass.MemorySpace.PSUM) as psZ,
                tc.tile_pool(name="psQ", bufs=1,
                             space=bass.MemorySpace.PSUM) as psQ,
                tc.tile_pool(name="erf", bufs=3) as epool,
            ):
                q = psQ.tile([128, 2 * B], F32)
                edg = bb[0:4, BB_EDG:BB_EDG + 128]
                for hh in range(2):
                    base = 0
                    for gel in GROUPS:
                        zt = psZ.tile([128, 1536], F32, tag="zt")
                        for h in range(gel // 512):
                            off = base + h * 512
                            rsrc = (R1[hh][:, off:off + 512] if off < RHEAD
                                    else R2[hh][:, off - RHEAD:off - RHEAD + 512])
                            nc.tensor.matmul(
                                zt[:, h * 512:(h + 1) * 512], edg, rsrc,
                                start=True, stop=True)
                        et = epool.tile([128, 1536], FP8, tag="et")
                        nc.scalar.activation(et[:, 0:gel], zt[:, 0:gel],
                                             AF.Tanh, bias=0.0, scale=ATAN)
                        for j in range(gel // 128):
                            c = (hh * HELEMS + base) // 128 + j
                            nc.tensor.matmul(q[:, 2 * c:2 * c + 2],
                                             et[:, j * 128:(j + 1) * 128],
                                             uv[:], start=True, stop=True)
                        base += gel

                # tail: part = sum_cols (sqw*(xqc - q0 - 0.992*q1))^2
                q2 = q[:].rearrange("p (c two) -> p c two", two=2)
                t0 = work.tile([128, B], F32)
                nc.vector.scalar_tensor_tensor(t0[:], q2[:, :, 1], -0.9921875,
                                               f128[:, 0:B], op0=MULT, op1=ADD)
                e1 = work.tile([128, B], F32)
                nc.vector.tensor_tensor(e1[:], t0[:], q2[:, :, 0], SUB)
                dw = work.tile([128, B], F32)
                nc.vector.tensor_tensor(dw[:], e1[:], f128[:, B:2 * B], MULT)
                dw2 = work.tile([128, B], F32)
                part = work.tile([128, 1], F32)
                nc.vector.scalar_tensor_tensor(dw2[:], dw[:], 1.0, dw[:],
                                               op0=BYP, op1=MULT,
                                               accum_out=part[:])
                nc.sync.dma_start(d_part.ap()[:], part[:])

    nc.compile()
    return nc


def host_prep(x, t, noise, W1, b1, W2, b2):
    """Build the per-core in_maps (host-side packing + tiny per-row math)."""
    f32 = np.float32
    tv = t[:, 0].astype(f32)
    gamma = (1.0 - np.power(f32(SIGMA1), f32(2.0) * tv)).astype(f32)
    low = tv < TMIN
    mf = np.where(low, f32(0.0), f32(1.0)).astype(f32)
    gsafe = np.where(gamma > 0, gamma, f32(1.0)).astype(f32)
    r = np.sqrt((1.0 - gsafe) / gsafe).astype(f32)
    rsafe = np.where(r > 0, r, f32(1.0)).astype(f32)
    bv = ((1.0 - gamma) * mf).astype(f32)
    rm = (r * mf).astype(f32)
    cexp = np.where(low, f32(1.0 / np.sqrt(2.0)),
                    (1.0 / (rsafe * np.sqrt(2.0))).astype(f32)).astype(f32)
    sqw = np.power(f32(SIGMA1), -tv).astype(f32)

    xT = np.ascontiguousarray(x.T, dtype=f32)
    nT = np.ascontiguousarray(noise.T, dtype=f32)
    g2 = (gamma * (1.0 - gamma)).astype(f32)
    muT8 = np.ascontiguousarray(
        (xT * gamma[None, :] + nT * g2[None, :]).astype(f32)
        .reshape(8, 128, B).transpose(1, 0, 2).reshape(128, 8 * B)
        .astype(F8NP))

    # w1m[p, (m*8+k)*128 + c] = W1[k*128+p, m*128+c]
    w1f = W1[:D].astype(f32).reshape(8, 128, 16, 128)
    w1m = np.ascontiguousarray(
        w1f.transpose(1, 2, 0, 3).reshape(128, 16 * 8 * 128).astype(F8NP))

    # bb blob
    bbv = np.zeros((4, BB_W), dtype=BFNP)
    bbv[0, BB_TV:BB_TV + B] = tv.astype(BFNP)
    bbv[1, BB_TV:BB_TV + B] = BFNP(1.0)
    bbv[0, BB_W1T:BB_W1T + H] = W1[D].astype(BFNP)
    bbv[1, BB_W1T:BB_W1T + H] = b1.astype(BFNP)
    e = (2.0 * np.arange(1, K) / K - 1.0).astype(f32)  # 127 edges, bf16-exact
    bbv[0, BB_EDG:BB_EDG + 127] = e.astype(BFNP)
    bbv[1, BB_EDG:BB_EDG + 127] = e.astype(BFNP)
    bbv[2, BB_EDG:BB_EDG + 127] = BFNP(-1.0)
    bbv[3, BB_EDG:BB_EDG + 127] = BFNP(-1.0)
    bbv[0, BB_ONE:BB_ONE + B] = BFNP(1.0)

    # f64 blob (per-batch broadcasts, tiled x2 over dh)
    f64v = np.zeros((64, F64_W), dtype=f32)
    for off, v in ((FO_MF, mf), (FO_BV, bv), (FO_RM, rm), (FO_CE, cexp)):
        f64v[:, off:off + 512] = np.tile(v, 2)[None, :]

    uvec = np.zeros((128, 2), dtype=F8NP)
    uvec[:127, 0] = F8NP(-1.0 / K)
    uvec[126, 1] = F8NP(0.5)

    # q layout index math (dh-major): flat = c*128 + p ;
    # g = dh*16384 + p64*256 + b, d_local = dh*64 + p64
    p_idx = np.arange(128)[:, None]
    c_idx = np.arange(B)[None, :]
    g = c_idx * 128 + p_idx
    dh = g // HELEMS
    p64 = (g % HELEMS) // B
    b_i = g % B
    d_l = dh * 64 + p64
    sqwq = np.ascontiguousarray(sqw[b_i], dtype=f32)

    def to64(a128):
        # [128 d, 256 b] -> [64 p, 2, 256] with [p, dh, b] = a[dh*64+p, b]
        return np.ascontiguousarray(
            a128.reshape(2, 64, B).transpose(1, 0, 2).reshape(64, 2 * B))

    in_maps = []
    for i in range(NCORES):
        cols = np.concatenate([np.arange(i * DSL, (i + 1) * DSL),
                               1024 + np.arange(i * DSL, (i + 1) * DSL)])
        # w2m[p, k*256 + c] = W2[k*128+p, cols[c]]
        w2m = np.ascontiguousarray(
            W2[:, cols].astype(f32).reshape(16, 128, 2 * DSL)
            .transpose(1, 0, 2).reshape(128, 16 * 2 * DSL).astype(F8NP))
        bbi = bbv.copy()
        bbi[0, BB_B2:BB_B2 + 2 * DSL] = b2[cols].astype(BFNP)
        f64i = f64v.copy()
        f64i[:, FO_XS:FO_XS + 512] = to64(xT[i * DSL:(i + 1) * DSL])
        f64i[:, FO_NS:FO_NS + 512] = to64(nT[i * DSL:(i + 1) * DSL])
        f128 = np.empty((128, 2 * B), dtype=f32)
        f128[:, 0:B] = x[b_i, i * DSL + d_l].astype(f32) + f32(C127)
        f128[:, B:2 * B] = sqwq
        in_maps.append({
            "muT8": muT8, "w1m": w1m, "w2m": w2m, "bb": bbi,
            "f64": f64i, "f128": f128, "uv": uvec,
        })
    return in_maps


_nc_cache = {}


def get_nc(debug=False):
    if debug not in _nc_cache:
        _nc_cache[debug] = _build(debug)
    return _nc_cache[debug]


def run_on_cores(inputs, trace=False, debug=False, tmpdir=None):
    nc = get_nc(debug)
    in_maps = host_prep(**inputs)
    res = run_bass_kernel_spmd(nc, in_maps, core_ids=list(range(NCORES)),
                               trace=trace, tmpdir=tmpdir)
    total = np.float32(0.0)
    for i in range(NCORES):
        total += res.results[i]["part"].astype(np.float32).sum()
    loss = np.float32(-np.log(np.float32(SIGMA1)) * total / np.float32(B * D))
    return loss, res


def kernel(**inputs):
    inputs = {k: np.asarray(v) for k, v in inputs.items()}
    loss, _ = run_on_cores(inputs)
    return np.asarray(loss, dtype=np.float32)


# revision 10
# speedup vs baseline: 1.1010x; 1.0412x over previous
"""Trainium2 Bass kernel for nn_DiscretisedBNF (discretised BNF loss).

Math reduction used on device: the reference's (B, D, K=128) clamped-CDF
bin sum collapses (Abel summation) to

    pO[b,d] = -127/256 + sum_{k=1..127} u_k * erf(z_k),
    z_k = (e_k - mu_x) * inv,   e_k = 2k/128 - 1,
    u_k = -1/128 (k<127),  u_127 = 125/256,
    inv = 1 / (sigma_x * sqrt(2))

erf is approximated on device by tanh(1.20331*z) (minimax fit, max abs
err 0.019; end-to-end loss rel err ~1.6e-3 incl. all quantization) so
that the whole kernel uses a single ACT table set (exp_and_others has
exp, tanh and leaky_relu; erf would force a ~2.7us table switch between
the exp and the binning phase).

Sharding (8 cores, full inputs in, full output out):
  - mm1 (mu_cat @ W1) replicated per core, fp8 DoubleRow (2 k-subtiles
    per matmul), with the t-row and b1 folded in as a K=2 bf16 matmul,
  - W2 column-sharded: core i owns output columns {i*128..} (mu_eps)
    and {1024+i*128..} (ln_sigma); mm2 fp8 DoubleRow + b2 ones-row,
  - binning data-parallel over the same d-slice: 32768 elements/core,
  - per-core output: 128 partial sums of sigma1^{-2t}*(x-pO)^2; host
    reduces and scales.

Inputs are host-packed into a few large SBUF-layout blobs so the input
pipe is ~15 large DMAs split across both HWDGE queues instead of ~40
small serialized ones. Element order for binning is dh-major
(g = dh*16384 + p64*256 + b, d_local = dh*64 + p64) so each half of the
prep (driven by one mu_eps/ln_sig half of mm2) feeds a contiguous run
of bin groups, letting ACT start tanh right behind mm2.
"""

import sys

sys.path.insert(0, "/opt/trn_rl_repo")

import numpy as np
import ml_dtypes

import concourse.bass as bass
import concourse.tile as tile
from concourse import bacc, mybir
from concourse.alu_op_type import AluOpType
from concourse.bass_utils import run_bass_kernel_spmd

B, D, H, K = 256, 1024, 2048, 128
NCORES = 8
DSL = D // NCORES  # 128 d-columns per core
SIGMA1 = 0.02
TMIN = 1e-10
LEAK = 0.01
C127 = 127.0 / 256.0
ATAN = 1.2033141525242548  # tanh(ATAN*z) ~= erf(z)

F32 = mybir.dt.float32
BF16 = mybir.dt.bfloat16
FP8 = mybir.dt.float8e4
BFNP = ml_dtypes.bfloat16
F8NP = ml_dtypes.float8_e4m3

HELEMS = DSL // 2 * B          # 16384 elements per dh half
RHEAD = 18 * B                 # 4608 = 3 groups of 1536 (partitions 0:18)
GROUPS = [1536] * 10 + [1024]  # per-half group sizes (sum = 16384)

# bb blob column offsets (bf16, 4 partitions)
BB_TV = 0         # [0:2, 0:256]   row0 = t, row1 = ones
BB_W1T = 256      # [0:2, 256:2304] row0 = W1[D,:], row1 = b1
BB_EDG = 2304     # [0:4, 2304:2432] edge matrix
BB_B2 = 2432      # [0:1, 2432:2688] b2[cols]
BB_ONE = 2688     # [0:1, 2688:2944] ones
BB_W = 2944

# f64 blob column offsets (f32, 64 partitions; 512-wide = tiled x2 over dh)
FO_MF, FO_BV, FO_RM, FO_CE, FO_XS, FO_NS = 0, 512, 1024, 1536, 2048, 2560
F64_W = 3072


def _build(debug=False):
    nc = bacc.Bacc("TRN2", target_bir_lowering=False, debug=False,
                   num_devices=NCORES)

    d_muT = nc.dram_tensor("muT8", (128, 8 * B), FP8, kind="ExternalInput")
    d_w1 = nc.dram_tensor("w1m", (128, 16 * 8 * 128), FP8,
                          kind="ExternalInput")
    d_w2 = nc.dram_tensor("w2m", (128, 16 * 2 * DSL), FP8,
                          kind="ExternalInput")
    d_bb = nc.dram_tensor("bb", (4, BB_W), BF16, kind="ExternalInput")
    d_f64 = nc.dram_tensor("f64", (64, F64_W), F32, kind="ExternalInput")
    d_f128 = nc.dram_tensor("f128", (128, 2 * B), F32, kind="ExternalInput")
    d_uv = nc.dram_tensor("uv", (128, 1), BF16, kind="ExternalInput")
    d_part = nc.dram_tensor("part", (128, 1), F32, kind="ExternalOutput")

    MULT, ADD, SUB, BYP = (AluOpType.mult, AluOpType.add,
                           AluOpType.subtract, AluOpType.bypass)
    AF = mybir.ActivationFunctionType
    DR = mybir.MatmulPerfMode.DoubleRow

    with tile.TileContext(nc) as tc:
        with (
            tc.tile_pool(name="weights", bufs=1) as wpool,
            tc.tile_pool(name="work", bufs=1) as work,
            tc.tile_pool(name="stage", bufs=1) as stage,
        ):
            muT = wpool.tile([128, 8, B], FP8)
            w1s = [wpool.tile([128, 4, 8, 128], FP8, name=f"w1s{i}")
                   for i in range(4)]
            w2 = wpool.tile([128, 16, 2 * DSL], FP8)
            bb = wpool.tile([4, BB_W], BF16)
            f64 = wpool.tile([64, F64_W], F32)
            f128 = wpool.tile([128, 2 * B], F32)
            uv = wpool.tile([128, 1], BF16)
            hT = work.tile([128, 16, B], FP8)

            with (
                tc.tile_pool(name="psA", bufs=3,
                             space=bass.MemorySpace.PSUM) as psA,
                tc.tile_pool(name="psO", bufs=1,
                             space=bass.MemorySpace.PSUM) as psO,
            ):
                # ---- input DMAs: sync (HWDGE) + gpsimd (SWDGE) queues;
                # the scalar/ACT engine issues NO DMAs so activations are
                # never stuck behind DMA instructions in its FIFO
                nc.gpsimd.dma_start(bb[:], d_bb.ap()[:])
                nc.sync.dma_start(muT[:], d_muT.ap()[:])
                nc.gpsimd.dma_start(f64[:], d_f64.ap()[:])
                for s4 in range(4):  # 4 m-tiles (512KB) per slab
                    nc.sync.dma_start(
                        w1s[s4][:], d_w1.ap()[:, s4 * 4096:(s4 + 1) * 4096])
                nc.gpsimd.dma_start(uv[:], d_uv.ap()[:])
                nc.gpsimd.dma_start(w2[:], d_w2.ap()[:])
                nc.gpsimd.dma_start(f128[:], d_f128.ap()[:])

                # s = x*mf + (1-gamma)*mf*noise  (the mu/gamma term, masked)
                a1 = work.tile([64, 2, B], F32)
                nc.vector.tensor_tensor(
                    a1[:], f64[:, FO_XS:FO_XS + 512], f64[:, FO_MF:FO_MF + 512],
                    MULT)
                a2 = work.tile([64, 2, B], F32)
                nc.vector.tensor_tensor(
                    a2[:], f64[:, FO_NS:FO_NS + 512], f64[:, FO_BV:FO_BV + 512],
                    MULT)
                s = work.tile([64, 2, B], F32)
                nc.vector.tensor_tensor(s[:], a1[:], a2[:], ADD)
                # dummy exp: pull the exp_and_others ACT table load into
                # the mm1 window (tanh/exp later need no load)
                dum = work.tile([1, 1], F32)
                nc.scalar.activation(dum[:], bb[0:1, 0:1], AF.Exp,
                                     bias=0.0, scale=1.0)
                # HAM warm-up fillers: keep the PE streaming from the
                # moment muT lands so mm1 runs at 2.4GHz from the start
                for wf in range(12):
                    wt = psA.tile([128, B], F32, tag="warm", bufs=1)
                    nc.tensor.matmul(wt[:], muT[:, 0, 0:128],
                                     muT[:, wf % 8, :], start=True, stop=True)

                # ---- mm1: hT[m] = LeakyReLU(W1^T mu_cat^T) fp8 DoubleRow;
                # t-row and b1 folded in as a K=2 bf16 matmul
                for m in range(16):
                    ph = psA.tile([128, B], F32, tag="ph")
                    for j in range(4):
                        nc.tensor.matmul(
                            ph[:], w1s[m // 4][:, m % 4, 2 * j:2 * j + 2, :],
                            muT[:, 2 * j:2 * j + 2, :],
                            start=(j == 0), stop=False, perf_mode=DR)
                    ms = slice(BB_W1T + m * 128, BB_W1T + (m + 1) * 128)
                    nc.tensor.matmul(ph[:], bb[0:2, ms], bb[0:2, BB_TV:BB_TV + B],
                                     start=False, stop=True)
                    u = work.tile([128, B], F32, tag="lrelu_u", bufs=2)
                    nc.vector.tensor_copy(u[:], ph[:])
                    nc.vector.scalar_tensor_tensor(
                        hT[:, m, :], u[:], LEAK, u[:],
                        op0=MULT, op1=AluOpType.max)

                # ---- mm2: po[mo] = W2[:,cols]^T hT + b2, fp8 DoubleRow.
                # Order (2,0,3,1): lnsig/mu_eps for dh=0 first so half-a
                # prep + binning launch while mm2 finishes half b.
                po = {}
                for mo in (2, 0, 3, 1):
                    pt = psO.tile([64, B], F32, tag=f"po{mo}")
                    po[mo] = pt
                    mos = slice(mo * 64, (mo + 1) * 64)
                    for j in range(8):
                        nc.tensor.matmul(pt[:], w2[:, 2 * j:2 * j + 2, mos],
                                         hT[:, 2 * j:2 * j + 2, :],
                                         start=(j == 0), stop=False,
                                         perf_mode=DR)
                    b2s = slice(BB_B2 + mo * 64, BB_B2 + (mo + 1) * 64)
                    nc.tensor.matmul(pt[:], bb[0:1, b2s],
                                     bb[0:1, BB_ONE:BB_ONE + B],
                                     start=False, stop=True)

                # ---- binning prep, per dh half -------------------------
                QT = [stage.tile([64, 4, B], BF16, name=f"QT{h}")
                      for h in range(2)]
                R1 = [stage.tile([4, RHEAD], BF16, name=f"R1{h}")
                      for h in range(2)]
                R2 = [stage.tile([4, HELEMS - RHEAD], BF16, name=f"R2{h}")
                      for h in range(2)]
                flat_engines = [nc.sync, nc.gpsimd]

                def flatten_row(hh, r):
                    for si, (p0, p1) in enumerate([(0, 18), (18, 64)]):
                        eng = flat_engines[(r + si) % 2]
                        dst = (R1[hh][r:r + 1, :] if si == 0 else
                               R2[hh][r:r + 1, :])
                        eng.dma_start(dst, QT[hh][p0:p1, r, :])

                warmn = [0]

                def warm_mm(dep_q, dep_r):
                    w = psA.tile([128, B], F32, tag="warm", bufs=1)
                    nc.tensor.matmul(w[:], QT[dep_q][:, dep_r, 0:128],
                                     QT[dep_q][:, dep_r, :],
                                     start=True, stop=True)
                    warmn[0] += 1

                inv = [None, None]
                for hh in range(2):
                    hs = slice(hh * B, (hh + 1) * B)
                    lnm = work.tile([64, B], F32, tag=f"lnm{hh}")
                    nc.vector.tensor_tensor(lnm[:], po[2 + hh][:],
                                            f64[:, FO_MF + hh * B:FO_MF + hh * B + B],
                                            MULT)
                    ei = work.tile([64, B], F32, tag=f"ei{hh}")
                    nc.scalar.activation(ei[:], lnm[:], AF.Exp, bias=0.0,
                                         scale=-1.0)
                    iv = work.tile([64, B], F32, tag=f"inv{hh}")
                    inv[hh] = iv
                    nc.vector.tensor_tensor(
                        iv[:], ei[:], f64[:, FO_CE + hh * B:FO_CE + hh * B + B],
                        MULT)
                    nc.vector.tensor_copy(QT[hh][:, 0, :], iv[:])   # ih
                    flatten_row(hh, 0)
                    nc.vector.tensor_tensor(QT[hh][:, 1, :], iv[:],
                                            QT[hh][:, 0, :], SUB)   # il
                    flatten_row(hh, 1)
                    warm_mm(hh, 0)
                    a4 = work.tile([64, B], F32, tag=f"a4{hh}")
                    nc.vector.tensor_tensor(
                        a4[:], f64[:, FO_RM + hh * B:FO_RM + hh * B + B],
                        po[hh][:], MULT)
                    mu_x = work.tile([64, B], F32, tag=f"mux{hh}")
                    nc.vector.tensor_tensor(mu_x[:], s[:, hh, :], a4[:], SUB)
                    mx = work.tile([64, B], F32, tag=f"mx{hh}")
                    nc.vector.tensor_tensor(mx[:], mu_x[:], iv[:], MULT)
                    nc.vector.tensor_copy(QT[hh][:, 2, :], mx[:])   # hi
                    flatten_row(hh, 2)
                    nc.vector.tensor_tensor(QT[hh][:, 3, :], mx[:],
                                            QT[hh][:, 2, :], SUB)   # lo
                    flatten_row(hh, 3)
                    warm_mm(hh, 2)

            # ---- binning main loop -------------------------------------
            with (
                tc.tile_pool(name="psZ", bufs=2,
                             space=bass.MemorySpace.PSUM) as psZ,
                tc.tile_pool(name="psQ", bufs=1,
                             space=bass.MemorySpace.PSUM) as psQ,
                tc.tile_pool(name="erf", bufs=3) as epool,
            ):
                q = psQ.tile([128, B], F32)
                edg = bb[0:4, BB_EDG:BB_EDG + 128]
                # (hh, base, gel) walk, z matmuls emitted one group ahead
                # of tanh/q so the PE never idles waiting on ACT
                walk = []
                for hh in range(2):
                    base = 0
                    for gel in GROUPS:
                        walk.append((hh, base, gel))
                        base += gel
                zts = {}

                def emit_z(gi):
                    hh, base, gel = walk[gi]
                    zt = psZ.tile([128, 1536], F32, tag="zt")
                    zts[gi] = zt
                    for h in range(gel // 512):
                        off = base + h * 512
                        rsrc = (R1[hh][:, off:off + 512] if off < RHEAD
                                else R2[hh][:, off - RHEAD:off - RHEAD + 512])
                        nc.tensor.matmul(
                            zt[:, h * 512:(h + 1) * 512], edg, rsrc,
                            start=True, stop=True)

                emit_z(0)
                for gi, (hh, base, gel) in enumerate(walk):
                    if gi + 1 < len(walk):
                        emit_z(gi + 1)
                    zt = zts.pop(gi)
                    et = epool.tile([128, 1536], FP8, tag="et")
                    nc.scalar.activation(et[:, 0:gel], zt[:, 0:gel],
                                         AF.Tanh, bias=0.0, scale=ATAN)
                    for j in range(gel // 128):
                        c = (hh * HELEMS + base) // 128 + j
                        nc.tensor.matmul(q[:, c:c + 1],
                                         et[:, j * 128:(j + 1) * 128],
                                         uv[:], start=True, stop=True)

                # tail: part = sum_cols (sqw*(xqc - q))^2
                e1 = work.tile([128, B], F32)
                nc.vector.tensor_tensor(e1[:], f128[:, 0:B], q[:], SUB)
                dw = work.tile([128, B], F32)
                nc.vector.tensor_tensor(dw[:], e1[:], f128[:, B:2 * B], MULT)
                dw2 = work.tile([128, B], F32)
                part = work.tile([128, 1], F32)
                nc.vector.scalar_tensor_tensor(dw2[:], dw[:], 1.0, dw[:],
                                               op0=BYP, op1=MULT,
                                               accum_out=part[:])
                nc.sync.dma_start(d_part.ap()[:], part[:])

    nc.compile()
    return nc


def host_prep(x, t, noise, W1, b1, W2, b2):
    """Build the per-core in_maps (host-side packing + tiny per-row math)."""
    f32 = np.float32
    tv = t[:, 0].astype(f32)
    gamma = (1.0 - np.power(f32(SIGMA1), f32(2.0) * tv)).astype(f32)
    low = tv < TMIN
    mf = np.where(low, f32(0.0), f32(1.0)).astype(f32)
    gsafe = np.where(gamma > 0, gamma, f32(1.0)).astype(f32)
    r = np.sqrt((1.0 - gsafe) / gsafe).astype(f32)
    rsafe = np.where(r > 0, r, f32(1.0)).astype(f32)
    bv = ((1.0 - gamma) * mf).astype(f32)
    rm = (r * mf).astype(f32)
    cexp = np.where(low, f32(1.0 / np.sqrt(2.0)),
                    (1.0 / (rsafe * np.sqrt(2.0))).astype(f32)).astype(f32)
    sqw = np.power(f32(SIGMA1), -tv).astype(f32)

    xT = np.ascontiguousarray(x.T, dtype=f32)
    nT = np.ascontiguousarray(noise.T, dtype=f32)
    g2 = (gamma * (1.0 - gamma)).astype(f32)
    muT8 = np.ascontiguousarray(
        (xT * gamma[None, :] + nT * g2[None, :]).astype(f32)
        .reshape(8, 128, B).transpose(1, 0, 2).reshape(128, 8 * B)
        .astype(F8NP))

    # w1m[p, (m*8+k)*128 + c] = W1[k*128+p, m*128+c]
    w1f = W1[:D].astype(f32).reshape(8, 128, 16, 128)
    w1m = np.ascontiguousarray(
        w1f.transpose(1, 2, 0, 3).reshape(128, 16 * 8 * 128).astype(F8NP))

    # bb blob
    bbv = np.zeros((4, BB_W), dtype=BFNP)
    bbv[0, BB_TV:BB_TV + B] = tv.astype(BFNP)
    bbv[1, BB_TV:BB_TV + B] = BFNP(1.0)
    bbv[0, BB_W1T:BB_W1T + H] = W1[D].astype(BFNP)
    bbv[1, BB_W1T:BB_W1T + H] = b1.astype(BFNP)
    e = (2.0 * np.arange(1, K) / K - 1.0).astype(f32)  # 127 edges, bf16-exact
    bbv[0, BB_EDG:BB_EDG + 127] = e.astype(BFNP)
    bbv[1, BB_EDG:BB_EDG + 127] = e.astype(BFNP)
    bbv[2, BB_EDG:BB_EDG + 127] = BFNP(-1.0)
    bbv[3, BB_EDG:BB_EDG + 127] = BFNP(-1.0)
    bbv[0, BB_ONE:BB_ONE + B] = BFNP(1.0)

    # f64 blob (per-batch broadcasts, tiled x2 over dh)
    f64v = np.zeros((64, F64_W), dtype=f32)
    for off, v in ((FO_MF, mf), (FO_BV, bv), (FO_RM, rm), (FO_CE, cexp)):
        f64v[:, off:off + 512] = np.tile(v, 2)[None, :]

    uvec = np.zeros((128, 1), dtype=BFNP)
    uvec[:126, 0] = BFNP(-1.0 / K)
    uvec[126, 0] = BFNP(125.0 / 256.0)  # exact in bf16

    # q layout index math (dh-major): flat = c*128 + p ;
    # g = dh*16384 + p64*256 + b, d_local = dh*64 + p64
    p_idx = np.arange(128)[:, None]
    c_idx = np.arange(B)[None, :]
    g = c_idx * 128 + p_idx
    dh = g // HELEMS
    p64 = (g % HELEMS) // B
    b_i = g % B
    d_l = dh * 64 + p64
    sqwq = np.ascontiguousarray(sqw[b_i], dtype=f32)

    def to64(a128):
        # [128 d, 256 b] -> [64 p, 2, 256] with [p, dh, b] = a[dh*64+p, b]
        return np.ascontiguousarray(
            a128.reshape(2, 64, B).transpose(1, 0, 2).reshape(64, 2 * B))

    in_maps = []
    for i in range(NCORES):
        cols = np.concatenate([np.arange(i * DSL, (i + 1) * DSL),
                               1024 + np.arange(i * DSL, (i + 1) * DSL)])
        # w2m[p, k*256 + c] = W2[k*128+p, cols[c]]
        w2m = np.ascontiguousarray(
            W2[:, cols].astype(f32).reshape(16, 128, 2 * DSL)
            .transpose(1, 0, 2).reshape(128, 16 * 2 * DSL).astype(F8NP))
        bbi = bbv.copy()
        bbi[0, BB_B2:BB_B2 + 2 * DSL] = b2[cols].astype(BFNP)
        f64i = f64v.copy()
        f64i[:, FO_XS:FO_XS + 512] = to64(xT[i * DSL:(i + 1) * DSL])
        f64i[:, FO_NS:FO_NS + 512] = to64(nT[i * DSL:(i + 1) * DSL])
        f128 = np.empty((128, 2 * B), dtype=f32)
        f128[:, 0:B] = x[b_i, i * DSL + d_l].astype(f32) + f32(C127)
        f128[:, B:2 * B] = sqwq
        in_maps.append({
            "muT8": muT8, "w1m": w1m, "w2m": w2m, "bb": bbi,
            "f64": f64i, "f128": f128, "uv": uvec,
        })
    return in_maps


_nc_cache = {}


def get_nc(debug=False):
    if debug not in _nc_cache:
        _nc_cache[debug] = _build(debug)
    return _nc_cache[debug]


def run_on_cores(inputs, trace=False, debug=False, tmpdir=None):
    nc = get_nc(debug)
    in_maps = host_prep(**inputs)
    res = run_bass_kernel_spmd(nc, in_maps, core_ids=list(range(NCORES)),
                               trace=trace, tmpdir=tmpdir)
    total = np.float32(0.0)
    for i in range(NCORES):
        total += res.results[i]["part"].astype(np.float32).sum()
    loss = np.float32(-np.log(np.float32(SIGMA1)) * total / np.float32(B * D))
    return loss, res


def kernel(**inputs):
    inputs = {k: np.asarray(v) for k, v in inputs.items()}
    loss, _ = run_on_cores(inputs)
    return np.asarray(loss, dtype=np.float32)
